# revision 15
# baseline (speedup 1.0000x reference)
"""Trainium2 Bass kernel for nn_AttentionBlock (sparse causal attention).

Math (reference):
  Omega[b,h,t,u] = sum_ij r'[b,t,i] Q[h,i,j] r'[b,u,j]      (then causal mask)
  r[b,t,i]       = sum_{h,u,j} Omega[b,h,t,u] E[h,i,j] r'[b,u,j]

Per (b,h) with R = r'[b] (T x n):
  K = R @ Q_h          (T x n)
  V = R @ E_h^T        (T x n)
  r_b += causal(K @ R^T) @ V
Computed flash-style with the chunked linear-attention decomposition:
for each row-chunk c (size C): out_c = K_c @ S_c + tri(K_c R_c^T) @ V_c,
S_c = sum_{c'<c} R_c'^T V_c'  (kept as a running PSUM accumulation).

Sharding: 8 cores = 2 batches x 4 head-groups (2 heads each). Each core
computes a partial r[b]; host sums the 4 partials per batch.
"""
import os
import sys
import types

sys.path.insert(0, "/opt/trn_rl_repo")

import numpy as np

# ---------------------------------------------------------------- constants
T = 2048          # n_t tokens
N = 256           # model dim n
C = 256           # row-chunk size
P = C // 128      # 128-row subtiles per chunk
NCHUNK = T // C   # 8
NH = 2            # heads per core
B = 2
H_TOT = 8
N_CORES = 8

DTYPE_NAME = os.environ.get("KDTYPE", "float32r")  # float32 | float32r | bfloat16


def _patch_tile_drain(max_waits=1):
    """walrus in this image accepts only 1 sync-wait per instruction; Tile's
    final drain aggregates one wait per live semaphore. Spread them over
    sequential SP nops (SP executes in order, so the net wait is identical)."""
    import concourse.tile as tile_mod
    from concourse.vector_clock import ScopedClock
    from concourse import mybir

    if getattr(tile_mod.TileContext, "_drain_patched", False):
        return

    def _drain_and_barrier(self, tick_clock, wait_clock):
        carrier = self.nc.sync.nop()
        wait_clock.add_sem_waits(
            carrier.ins, ScopedClock({None: tick_clock.global_clock})
        )
        waits = list(carrier.ins.sync_info.on_wait or [])
        if len(waits) > max_waits:
            carrier.ins.sync_info.on_wait = waits[:max_waits]
            for i in range(max_waits, len(waits), max_waits):
                extra = self.nc.sync.nop()
                if extra.ins.sync_info is None:
                    extra.ins.sync_info = mybir.SyncInfo(on_wait=[], on_update=[])
                extra.ins.sync_info.on_wait = waits[i : i + max_waits]
        self.nc.sync.drain()

        self.nc.all_engine_barrier()
        assert self.sems is not None
        popped = self.nc._tile_sem_poison_stack.pop()
        assert popped is self._sem_poison
        self.nc.clear_and_free_semaphores(list(self.sems.allocated().values()))
        self.nc.all_engine_barrier()

    tile_mod.TileContext._drain_and_barrier = _drain_and_barrier
    tile_mod.TileContext._drain_patched = True


def _split_multi_waits(nc, max_waits=1):
    """walrus in this image accepts only `max_waits` sync-wait commands per
    TPB instruction. Hoist extra waits onto NoOps inserted just before the
    instruction on the same engine (engine streams are in-order, so waiting
    on a preceding NoOp is equivalent). DMA descriptors are left untouched."""
    from concourse import mybir

    ctr = 0
    for f in nc.m.functions:
        for bb in f.blocks:
            new_insts = []
            changed = False
            for inst in bb.instructions:
                si = inst.sync_info
                waits = list(si.on_wait) if (si and si.on_wait) else []
                if len(waits) > max_waits:
                    if "DMA" in type(inst).__name__:
                        # keep DMA-queue waits on the descriptor; hoist
                        # engine-producer waits onto the issuing engine
                        keep = [
                            w
                            for w in waits
                            if (w.ant_name or "").startswith("DMA")
                        ][:max_waits]
                        if not keep:
                            keep = waits[len(waits) - max_waits :]
                        hoist = [w for w in waits if w not in keep]
                    else:
                        hoist = waits[: len(waits) - max_waits]
                        keep = waits[len(waits) - max_waits :]
                    si.on_wait = keep
                    for j in range(0, len(hoist), max_waits):
                        ctr += 1
                        nop = mybir.InstNoOp(name=f"waitnop-{ctr}", ins=[], outs=[])
                        nop.engine = inst.engine
                        nop.sync_info = mybir.SyncInfo(
                            on_wait=hoist[j : j + max_waits], on_update=[]
                        )
                        new_insts.append(nop)
                    changed = True
                new_insts.append(inst)
            if changed:
                try:
                    bb.instructions[:] = new_insts
                except TypeError:
                    bb.instructions = new_insts
    return ctr


def _dt(name):
    from concourse import mybir

    return {
        "float32": mybir.dt.float32,
        "float32r": mybir.dt.float32r,
        "bfloat16": mybir.dt.bfloat16,
    }[name]


def build_nc(dtype_name=DTYPE_NAME):
    """One SPMD program; per-core data differs via in_maps."""
    import concourse.bass as bass
    import concourse.tile as tile
    from concourse import mybir

    _patch_tile_drain()

    D = _dt(dtype_name)

    def mmcast(ap):
        return ap

    nc = bass.Bass()
    Rd = nc.dram_tensor("R", [T, N], D, kind="ExternalInput")
    RTd = nc.dram_tensor("RT", [N, T], D, kind="ExternalInput")
    Qd = nc.dram_tensor("Qm", [NH, N, N], D, kind="ExternalInput")
    ETd = nc.dram_tensor("ETm", [NH, N, N], D, kind="ExternalInput")
    Md = nc.dram_tensor("mask", [128, 3 * 128], D, kind="ExternalInput")
    OUTd = nc.dram_tensor("out", [T, N], mybir.dt.float32, kind="ExternalOutput")

    from contextlib import ExitStack

    with tile.TileContext(nc) as tc, ExitStack() as ctx:
        consts = ctx.enter_context(tc.tile_pool(name="consts", bufs=1))
        work = ctx.enter_context(tc.tile_pool(name="work", bufs=2))
        spool = ctx.enter_context(tc.tile_pool(name="spool", bufs=1))
        psum = ctx.enter_context(tc.tile_pool(name="psum", bufs=1, space="PSUM"))

        R_sb = consts.tile([128, T // 128, N], D)      # [u128, tsub, j]
        RT_sb = consts.tile([128, 2, T], D)            # [n128, nhalf, t]
        Q_sb = consts.tile([128, NH, 2, N], D)         # [i128, h, ihalf, j]
        ET_sb = consts.tile([128, NH, 2, N], D)        # [j128, h, jhalf, i]
        M_sb = consts.tile([128, 3 * 128], D)
        OUT_sb = consts.tile([128, T // 128, N], mybir.dt.float32)

        nc.sync.dma_start(out=R_sb, in_=Rd.rearrange("(c p) j -> p c j", p=128))
        nc.sync.dma_start(out=RT_sb, in_=RTd.rearrange("(a p) t -> p a t", p=128))
        nc.sync.dma_start(out=Q_sb, in_=Qd.rearrange("h (a p) j -> p h a j", p=128))
        nc.sync.dma_start(out=ET_sb, in_=ETd.rearrange("h (a p) i -> p h a i", p=128))
        nc.sync.dma_start(out=M_sb, in_=Md[:, :])

        outR = OUTd.rearrange("(c p) j -> p c j", p=128)

        # persistent S (prefix sums) per head: [j, i] packed [128, jhalf*256]
        S_sb = [
            spool.tile([128, 512], D, tag=f"S_sb{h}", name=f"S_sb{h}")
            for h in range(NH)
        ]

        for c in range(NCHUNK):
            out_ps = psum.tile([128, 512], mybir.dt.float32, tag="out_ps", bufs=1)
            out_first = True  # first matmul of this chunk's out_ps group
            for h in range(NH):
                # ---- A: KT[j, t_local] = Q_h^T @ R_c^T; packed [:, jh*256:+256]
                KT_ps = psum.tile([128, 512], mybir.dt.float32, tag="KT_ps", bufs=2)
                for jh in range(2):
                    for a in range(2):  # i-halves (contraction)
                        nc.tensor.matmul(
                            KT_ps[:, jh * 256 : jh * 256 + 256],
                            mmcast(Q_sb[:, h, a, jh * 128 : jh * 128 + 128]),
                            mmcast(RT_sb[:, a, c * C : c * C + C]),
                            start=(jh == 0 and a == 0),
                            stop=(jh == 1 and a == 1),
                        )
                KT_sb = work.tile([128, 512], D, tag="KT_sb")
                nc.scalar.copy(out=KT_sb, in_=KT_ps)

                # ---- B: V[u, i] = R_c @ E_h^T; packed [:, p*256:+256]
                V_ps = psum.tile([128, 512], mybir.dt.float32, tag="V_ps", bufs=2)
                for p in range(P):
                    for a in range(2):  # j-halves (contraction)
                        nc.tensor.matmul(
                            V_ps[:, p * 256 : p * 256 + 256],
                            mmcast(
                                RT_sb[:, a, c * C + p * 128 : c * C + p * 128 + 128]
                            ),
                            mmcast(ET_sb[:, h, a, :]),
                            start=(p == 0 and a == 0),
                            stop=(p == P - 1 and a == 1),
                        )
                V_sb = work.tile([128, 512], D, tag="V_sb")
                nc.scalar.copy(out=V_sb, in_=V_ps)

                # ---- D: OmT[u, t_local] diag-chunk blocks, trailing columns
                # packed: p0 cols[0:256] (t_local 0..256), p1 cols[256:384]
                # (t_local 128..256)
                Om_ps = psum.tile([128, 384], mybir.dt.float32, tag="Om_ps", bufs=1)
                for p in range(P):
                    off = 256 * p
                    ncols = C - 128 * p
                    for a in range(2):  # j-halves
                        nc.tensor.matmul(
                            Om_ps[:, off : off + ncols],
                            mmcast(
                                RT_sb[:, a, c * C + p * 128 : c * C + p * 128 + 128]
                            ),
                            mmcast(KT_sb[:, a * 256 + p * 128 : a * 256 + 256]),
                            start=(p == 0 and a == 0),
                            stop=(p == P - 1 and a == 1),
                        )
                Om_sb = work.tile([128, 384], D, tag="Om_sb")
                nc.vector.tensor_mul(out=Om_sb, in0=Om_ps, in1=M_sb)

                # ---- C: main term out[t_sub, i] += KT^T @ S   (prefix < c)
                if c > 0:
                    for p in range(P):
                        for a in range(2):  # j-halves
                            nc.tensor.matmul(
                                out_ps[:, p * 256 : p * 256 + 256],
                                mmcast(
                                    KT_sb[:, a * 256 + p * 128 : a * 256 + p * 128 + 128]
                                ),
                                mmcast(S_sb[h][:, a * 256 : a * 256 + 256]),
                                start=out_first,
                                stop=False,
                            )
                            out_first = False

                # ---- E: diag term out[t_sub q, i] += OmT[p][:, q]^T @ V[p]
                for q in range(P):
                    for p in range(q + 1):
                        off = 256 * p + (q - p) * 128
                        nc.tensor.matmul(
                            out_ps[:, q * 256 : q * 256 + 256],
                            mmcast(Om_sb[:, off : off + 128]),
                            mmcast(V_sb[:, p * 256 : p * 256 + 256]),
                            start=out_first,
                            stop=(h == NH - 1 and q == P - 1 and p == q),
                        )
                        out_first = False

                # ---- F: dS[j, i] = R_c^T @ V, then S += dS  (skip last chunk)
                if c < NCHUNK - 1:
                    dS_ps = psum.tile(
                        [128, 512], mybir.dt.float32, tag="dS_ps", bufs=2
                    )
                    for a in range(2):  # j-halves (output partition)
                        for p in range(P):
                            nc.tensor.matmul(
                                dS_ps[:, a * 256 : a * 256 + 256],
                                mmcast(
                                    R_sb[:, P * c + p, a * 128 : a * 128 + 128]
                                ),
                                mmcast(V_sb[:, p * 256 : p * 256 + 256]),
                                start=(a == 0 and p == 0),
                                stop=(a == 1 and p == P - 1),
                            )
                    if c == 0:
                        nc.vector.tensor_copy(out=S_sb[h], in_=dS_ps)
                    else:
                        nc.vector.tensor_add(out=S_sb[h], in0=S_sb[h], in1=dS_ps)

            # ---- evacuate chunk output and stream to HBM
            nc.scalar.copy(
                out=OUT_sb[:, P * c : P * c + P, :],
                in_=out_ps.rearrange("p (a j) -> p a j", a=P),
            )
            nc.sync.dma_start(
                out=outR[:, P * c : P * c + P, :],
                in_=OUT_sb[:, P * c : P * c + P, :],
            )

    _split_multi_waits(nc)
    return nc


def _np_dt(dtype_name):
    if dtype_name == "bfloat16":
        import ml_dtypes

        return ml_dtypes.bfloat16
    return np.float32


def _round_f32r(x):
    """Round f32 to a float32r-representable value (exact bf16 hi+lo sum)."""
    import ml_dtypes

    x = np.asarray(x, dtype=np.float32)
    hi = x.astype(ml_dtypes.bfloat16).astype(np.float32)
    lo = (x - hi).astype(ml_dtypes.bfloat16).astype(np.float32)
    return hi + lo


def make_in_maps(r_prime, Q, E, dtype_name=DTYPE_NAME):
    """Host-side sharding: core k -> batch k//4, heads [2*(k%4), 2*(k%4)+1]."""
    ndt = _np_dt(dtype_name)
    if dtype_name == "float32r":
        def conv(x):
            return _round_f32r(np.ascontiguousarray(x))
    else:
        def conv(x):
            return np.ascontiguousarray(x).astype(ndt)

    tri = np.tril(np.ones((128, 128), dtype=np.float32)).T  # keep u <= t
    mask = np.concatenate([tri, np.ones((128, 128), np.float32), tri], axis=1)

    in_maps = []
    for k in range(N_CORES):
        b = k // 4
        h0 = 2 * (k % 4)
        in_maps.append(
            {
                "R": conv(r_prime[b]),
                "RT": conv(r_prime[b].T),
                "Qm": conv(Q[h0 : h0 + 2]),
                "ETm": conv(E[h0 : h0 + 2].transpose(0, 2, 1)),
                "mask": conv(mask),
            }
        )
    return in_maps


_NC_CACHE = {}


def get_nc(dtype_name=DTYPE_NAME):
    if dtype_name not in _NC_CACHE:
        _NC_CACHE[dtype_name] = build_nc(dtype_name)
    return _NC_CACHE[dtype_name]


def kernel(r_prime, Q, E):
    from concourse import bass2jax

    nc = get_nc()
    in_maps = make_in_maps(r_prime, Q, E)
    results = bass2jax.run_bass_via_pjrt(nc, in_maps, n_cores=N_CORES)
    out = np.zeros((B, T, N), dtype=np.float32)
    for k in range(N_CORES):
        out[k // 4] += results[k]["out"]
    return out
